# revision 11
# baseline (speedup 1.0000x reference)
"""EME loss kernel for Trainium2, 8 NeuronCores, pure data-parallel.

Math (matches the jax reference):
  y_pred [32, 3, 1024, 1024] f32; 8x8 non-overlapping window max/min pooling;
  mask = (max != min); vals = 20*ln(max/(min+1e-4)) where mask else 0;
  per_batch = sum(vals)/(1024*1024)*64; out = mean(per_batch)  -> f32 scalar.

Sharding: batch across 8 cores (4 batches = 12 images of 1024x1024 per core).
Device computes per-partition partial sums of (ln(max+eps) - ln(min+eps));
host combines: out = total * 20 * 64 / 2^20 / 32.  (The (max != min) mask is
dropped: a constant 8x8 window cannot occur with continuous uniform inputs;
adding eps to max as well changes ln(max) by <2e-4 relative.)

Layout: a 1024x1024 f32 image viewed as [128, 8192] puts one window-row
(8 image rows, 32KB contiguous) on each partition; per-partition free layout
is idx = r*1024 + w*8 + j (r = row in window, w = window, j = col in window).
SWDGE DMA casts fp32 -> bf16 inline, so DVE runs in 2x mode throughout.

Schedule (DMA floor: 48MiB read at the ~433GB/s 16-engine SDMA rate ~111us;
DVE total ~118us of bf16 2x TTs -- the two are balanced, so the schedule
minimizes DVE start lag and end jam):
  - Image 0 first, loaded in 1MiB quarters with a fold-chain tree: DVE starts
    ~13us in. Image 1 next in halves with a whole-image tree.
  - Images 2..9 as PAIRS, each loaded by ONE 8MiB transfer (per-partition two
    32KB runs at 4MiB stride; fewer transfers avoided a per-transfer SDMA
    straggler penalty seen with 2MiB chunks). Each reduction level is one TT
    per tree with a multi-dim AP spanning both images (12 TTs, ~18.6us vs
    ~21.4us unfused), just under the 19.35us pair DMA cadence.
  - Image 10 in halves, image 11 in quarters with the fold chain, so the DVE
    work that depends on the final DMA chunk is only ~4.6us -> short tail.
  - ACT does ln with free accumulation into per-image/pair columns of parts
    matrices; one subtract/reduce/matmul(ones) collapse at the end gives a
    single-descriptor [1,1] output DMA.
"""
import numpy as np
import concourse.bass as bass
import concourse.mybir as mybir
import concourse.tile as tile
from concourse.bass_utils import run_bass_kernel_spmd

_N_CORES = 8
_B, _C, _H, _W = 32, 3, 1024, 1024
_IMGS_PER_CORE = (_B // _N_CORES) * _C  # 12
_WIN = 8
_EPS = 1e-4

_NC_CACHE = {}
LAST_RESULTS = None  # BassKernelResults of the most recent run (for test.py)


def _split_excess_waits(nc, max_waits=1):
    """This walrus build rejects >2 sync-waits on one CTRL instruction (the
    Tile exit drain collects one wait per active logical proc). Move excess
    waits onto preceding NoOps on the same engine."""
    for func in nc.m.functions:
        for bb in func.blocks:
            insts = bb.instructions
            out_insts = []
            changed = False
            for ins in insts:
                si = getattr(ins, "sync_info", None)
                if si is not None and si.on_wait and len(si.on_wait) > max_waits:
                    waits = list(si.on_wait)
                    head, tail = waits[:-max_waits], waits[-max_waits:]
                    for j in range(0, len(head), max_waits):
                        nop = mybir.InstNoOp(name=f"{ins.name}-wsplit{j}", ins=[], outs=[])
                        nop.engine = ins.engine
                        nop.sync_info = mybir.SyncInfo(
                            on_wait=head[j:j + max_waits], on_update=[])
                        out_insts.append(nop)
                    ins.sync_info = mybir.SyncInfo(on_wait=tail, on_update=si.on_update)
                    changed = True
                out_insts.append(ins)
            if changed:
                bb.instructions = out_insts


def _light_drain_and_barrier(self, tick_clock, wait_clock):
    """TileContext exit ceremony minus the trailing all-engine barrier
    (drain already waits on the global clock; NEFF completion waits on all
    engine programs regardless). Saves a few us of kernel-exit time."""
    from concourse.vector_clock import ScopedClock
    drain_inst = self.nc.sync.drain()
    wait_clock.add_sem_waits(drain_inst.ins,
                             ScopedClock({None: tick_clock.global_clock}))
    self.nc.all_engine_barrier()
    popped = self.nc._tile_sem_poison_stack.pop()
    assert popped is self._sem_poison
    # skip clear_and_free_semaphores: NRT resets engine/sem state per
    # execution, and nothing runs after this context in the program
    self.nc._state.prepend_free_semaphores(
        [s.num if hasattr(s, "num") else s for s in self.sems.allocated().values()])


def _build():
    F32 = mybir.dt.float32
    BF16 = mybir.dt.bfloat16
    MAX = mybir.AluOpType.max
    MIN = mybir.AluOpType.min
    LN = mybir.ActivationFunctionType.Ln
    OPS = ((0, MAX), (1, MIN))  # (tree index, op); tree 0 = max, tree 1 = min

    nc = bass.Bass()
    y = nc.declare_dram_parameter("y", [_IMGS_PER_CORE, _H, _W], F32, isOutput=False)
    out = nc.declare_dram_parameter("out", [1, 1], F32, isOutput=True)

    n_cols = 8  # parts columns: img0, img1, 4 pairs, img10, img11

    tile.TileContext._drain_and_barrier = _light_drain_and_barrier
    with tile.TileContext(nc) as tc:
        with tc.tile_pool(name="pair", bufs=2) as pair_pool, \
             tc.tile_pool(name="taper", bufs=1) as taper_pool, \
             tc.tile_pool(name="work", bufs=1) as work_pool, \
             tc.tile_pool(name="stat", bufs=2) as stat_pool, \
             tc.tile_pool(name="accp", bufs=1) as acc_pool, \
             tc.tile_pool(name="psum", bufs=1, space="PSUM") as psum_pool:
            parts_mx = acc_pool.tile([128, n_cols], F32, tag="pmx")
            parts_mn = acc_pool.tile([128, n_cols], F32, tag="pmn")
            # warm the SP HWDGE queue at kernel start so the final out-DMA
            # doesn't pay first-use latency on the completion semaphore
            warm = acc_pool.tile([1, 1], F32, tag="warm")
            nc.sync.dma_start(out=warm[:], in_=y[0, 0:1, 0:1])

            def load(dst_ap, img, lo, hi):
                src = y[img].rearrange("(p r) c -> p (r c)", p=128)
                nc.gpsimd.dma_start(out=dst_ap, in_=src[:, lo:hi])

            # front taper tile: img0 (quarters) | img1 (halves)
            Tf = taper_pool.tile([128, 16384], BF16, tag="Tt")
            for q in range(4):
                load(Tf[:, q * 2048:(q + 1) * 2048], 0, q * 2048, (q + 1) * 2048)
            load(Tf[:, 8192:12288], 1, 0, 4096)
            load(Tf[:, 12288:16384], 1, 4096, 8192)

            eps = acc_pool.tile([128, 1], F32, tag="eps")
            nc.gpsimd.memset(eps[:], _EPS)
            ones = acc_pool.tile([128, 1], F32, tag="ones")
            nc.gpsimd.memset(ones[:], 1.0)
            lnscr = acc_pool.tile([128, 256], F32, tag="lnscr")  # ACT out sink

            def ln_accum(src_ap, col, t):
                parts = parts_mx if t == 0 else parts_mn
                nc.scalar.activation(lnscr[:, 0:src_ap.shape[-1]], src_ap, LN,
                                     bias=eps[:], accum_out=parts[:, col:col + 1])

            def hlevels(cur2, seg, res2, t, op):
                """j-direction 8->4->2->1 for one tree. cur2 holds `seg`
                window-column groups of 1024; res2 gets seg*128 results at
                offset t*seg*128. Scratch tags sized for seg=2, sliced."""
                h4 = work_pool.tile([128, 1024], BF16, tag=f"h4t{t}")
                cj = cur2.rearrange("p (s w j) -> p s w j", s=seg, j=8)
                nc.vector.tensor_tensor(
                    out=h4[:, 0:seg * 512].rearrange("p (s w j) -> p s w j",
                                                     s=seg, j=4),
                    in0=cj[:, :, :, 0:4], in1=cj[:, :, :, 4:8], op=op)
                h2 = work_pool.tile([128, 512], BF16, tag=f"h2t{t}")
                hj = h4[:, 0:seg * 512].rearrange("p (s w j) -> p s w j",
                                                  s=seg, j=4)
                nc.vector.tensor_tensor(
                    out=h2[:, 0:seg * 256].rearrange("p (s w j) -> p s w j",
                                                     s=seg, j=2),
                    in0=hj[:, :, :, 0:2], in1=hj[:, :, :, 2:4], op=op)
                rj = h2[:, 0:seg * 256].rearrange("p (s w j) -> p s w j",
                                                  s=seg, j=2)
                nc.vector.tensor_tensor(
                    out=res2[:, t * seg * 128:(t + 1) * seg * 128].rearrange(
                        "p (s w j) -> p s w j", s=seg, j=1),
                    in0=rj[:, :, :, 0:1], in1=rj[:, :, :, 1:2], op=op)

            def fold_img(Ts, res_t, col):
                """Quarter-granular fold-chain tree over Ts [128, 8192]
                (quarters = row pairs): each TT depends on at most one
                quarter plus the running fold."""
                for t, op in OPS:
                    s = work_pool.tile([128, 4096], BF16, tag="scr")
                    f = None
                    for q in range(4):
                        w = s[:, (q % 2) * 1024:((q % 2) + 1) * 1024]
                        nc.vector.tensor_tensor(
                            out=w, in0=Ts[:, q * 2048:q * 2048 + 1024],
                            in1=Ts[:, q * 2048 + 1024:(q + 1) * 2048], op=op)
                        if q == 0:
                            f = w
                        else:
                            nf = s[:, (2 + (q % 2)) * 1024:(3 + (q % 2)) * 1024]
                            nc.vector.tensor_tensor(out=nf, in0=f, in1=w, op=op)
                            f = nf
                    hlevels(f, 1, res_t, t, op)
                    ln_accum(res_t[:, t * 128:(t + 1) * 128], col, t)

            def tree_img(Ts, res_t, col):
                """Whole-image tree over Ts [128, 8192] (halves): L1 per
                half so each depends on one 2MiB chunk."""
                ab = work_pool.tile([128, 16384], BF16, tag="ab")
                for t, op in OPS:
                    base = t * 4096
                    nc.vector.tensor_tensor(out=ab[:, base:base + 2048],
                                            in0=Ts[:, 0:2048],
                                            in1=Ts[:, 2048:4096], op=op)
                    nc.vector.tensor_tensor(out=ab[:, base + 2048:base + 4096],
                                            in0=Ts[:, 4096:6144],
                                            in1=Ts[:, 6144:8192], op=op)
                c = work_pool.tile([128, 8192], BF16, tag="c")
                for t, op in OPS:
                    nc.vector.tensor_tensor(
                        out=c[:, t * 2048:(t + 1) * 2048],
                        in0=ab[:, t * 4096:t * 4096 + 2048],
                        in1=ab[:, t * 4096 + 2048:(t + 1) * 4096], op=op)
                cur = work_pool.tile([128, 4096], BF16, tag="cur")
                for t, op in OPS:
                    nc.vector.tensor_tensor(
                        out=cur[:, t * 1024:(t + 1) * 1024],
                        in0=c[:, t * 2048:t * 2048 + 1024],
                        in1=c[:, t * 2048 + 1024:(t + 1) * 2048], op=op)
                for t, op in OPS:
                    hlevels(cur[:, t * 1024:(t + 1) * 1024], 1, res_t, t, op)
                    ln_accum(res_t[:, t * 128:(t + 1) * 128], col, t)

            # ---- image 0 (fold) and image 1 (tree) from the front tile ----
            res0 = stat_pool.tile([128, 256], BF16, tag="res0")
            fold_img(Tf[:, 0:8192], res0, 0)
            res1 = stat_pool.tile([128, 256], BF16, tag="res1")
            tree_img(Tf[:, 8192:16384], res1, 1)

            # ---- images 2..9 as pairs: one 8MiB transfer, 12 fused TTs ----
            for k in range(4):
                T = pair_pool.tile([128, 16384], BF16, tag="T")
                src = y[2 * k + 2:2 * k + 4].rearrange(
                    "i (p r) c -> p i (r c)", p=128)
                nc.gpsimd.dma_start(
                    out=T[:].rearrange("p (i k) -> p i k", i=2), in_=src)
                Tv = T[:].rearrange("p (i b k) -> p i b k", i=2, b=2)
                ab = work_pool.tile([128, 16384], BF16, tag="ab")  # [mx | mn]
                for t, op in OPS:
                    abv = ab[:, t * 8192:(t + 1) * 8192].rearrange(
                        "p (i b k) -> p i b k", i=2, b=2)
                    nc.vector.tensor_tensor(out=abv, in0=Tv[:, :, :, 0:2048],
                                            in1=Tv[:, :, :, 2048:4096], op=op)
                c = work_pool.tile([128, 8192], BF16, tag="c")  # [mx | mn]
                for t, op in OPS:
                    av = ab[:, t * 8192:(t + 1) * 8192].rearrange(
                        "p (i b k) -> p i b k", i=2, b=2)
                    nc.vector.tensor_tensor(
                        out=c[:, t * 4096:(t + 1) * 4096].rearrange(
                            "p (i k) -> p i k", i=2),
                        in0=av[:, :, 0, :], in1=av[:, :, 1, :], op=op)
                cur = work_pool.tile([128, 4096], BF16, tag="cur")  # [mx | mn]
                for t, op in OPS:
                    cv = c[:, t * 4096:(t + 1) * 4096].rearrange(
                        "p (i k) -> p i k", i=2)
                    nc.vector.tensor_tensor(
                        out=cur[:, t * 2048:(t + 1) * 2048].rearrange(
                            "p (i k) -> p i k", i=2),
                        in0=cv[:, :, 0:1024], in1=cv[:, :, 1024:2048], op=op)
                res = stat_pool.tile([128, 512], BF16, tag="res")  # [mx | mn]
                for t, op in OPS:
                    hlevels(cur[:, t * 2048:(t + 1) * 2048], 2, res, t, op)
                ln_accum(res[:, 0:256], 2 + k, 0)
                ln_accum(res[:, 256:512], 2 + k, 1)

            # ---- images 10 (tree) and 11 (fold) from the back taper tile ----
            Tb = taper_pool.tile([128, 16384], BF16, tag="Tt")
            load(Tb[:, 0:4096], 10, 0, 4096)
            load(Tb[:, 4096:8192], 10, 4096, 8192)
            for q in range(4):
                load(Tb[:, 8192 + q * 2048:8192 + (q + 1) * 2048],
                     11, q * 2048, (q + 1) * 2048)
            res10 = stat_pool.tile([128, 256], BF16, tag="res10")
            tree_img(Tb[:, 0:8192], res10, 6)
            res11 = stat_pool.tile([128, 256], BF16, tag="res11")
            fold_img(Tb[:, 8192:16384], res11, 7)

            # ---- final combine ----
            diff = acc_pool.tile([128, n_cols], F32, tag="diff")
            nc.vector.tensor_tensor(out=diff[:], in0=parts_mx[:], in1=parts_mn[:],
                                    op=mybir.AluOpType.subtract)
            acc = acc_pool.tile([128, 1], F32, tag="acc")
            nc.vector.tensor_reduce(out=acc[:], in_=diff[:],
                                    axis=mybir.AxisListType.X,
                                    op=mybir.AluOpType.add)
            # collapse partitions with a 1x128 @ 128x1 matmul so the out-DMA
            # is a single descriptor
            pt = psum_pool.tile([1, 1], F32, tag="pt")
            nc.tensor.matmul(pt[:], acc[:], ones[:])
            total = acc_pool.tile([1, 1], F32, tag="total")
            nc.vector.tensor_copy(out=total[:], in_=pt[:])
            nc.sync.dma_start(out=out[:], in_=total[:])

    _split_excess_waits(nc)
    return nc


def _get_nc():
    if "nc" not in _NC_CACHE:
        _NC_CACHE["nc"] = _build()
    return _NC_CACHE["nc"]


def kernel(y_pred, winSize=8, _trace=False, **_ignored):
    global LAST_RESULTS
    assert int(winSize) == _WIN
    y = np.ascontiguousarray(np.asarray(y_pred, dtype=np.float32))
    assert y.shape == (_B, _C, _H, _W)
    per_core_b = _B // _N_CORES
    in_maps = [
        {"y": y[c * per_core_b:(c + 1) * per_core_b].reshape(_IMGS_PER_CORE, _H, _W)}
        for c in range(_N_CORES)
    ]
    nc = _get_nc()
    res = run_bass_kernel_spmd(nc, in_maps, list(range(_N_CORES)), trace=_trace)
    LAST_RESULTS = res
    total = np.sum([float(r["out"][0, 0]) for r in res.results])
    val = total * 20.0 * (_WIN * _WIN) / (_H * _W) / _B
    return np.float32(val)


# revision 14
# speedup vs baseline: 1.1287x; 1.1287x over previous
"""EME loss kernel for Trainium2, 8 NeuronCores, pure data-parallel.

Math (matches the jax reference):
  y_pred [32, 3, 1024, 1024] f32; 8x8 non-overlapping window max/min pooling;
  mask = (max != min); vals = 20*ln(max/(min+1e-4)) where mask else 0;
  per_batch = sum(vals)/(1024*1024)*64; out = mean(per_batch)  -> f32 scalar.

Sharding: batch across 8 cores (4 batches = 12 images of 1024x1024 per core).
Device computes per-partition partial sums of (ln(max+eps) - ln(min+eps));
host combines: out = total * 20 * 64 / 2^20 / 32.  (The (max != min) mask is
dropped: a constant 8x8 window cannot occur with continuous uniform inputs;
adding eps to max as well changes ln(max) by <2e-4 relative.)

Layout: a 1024x1024 f32 image viewed as [128, 8192] puts one window-row
(8 image rows, 32KB contiguous) on each partition; per-partition free layout
is idx = r*1024 + w*8 + j (r = row in window, w = window, j = col in window).
SWDGE DMA casts fp32 -> bf16 inline, so DVE runs in 2x mode throughout.

Schedule (DMA floor: 48MiB read at the ~433GB/s 16-engine SDMA rate ~111us;
DVE total ~118us of bf16 2x TTs -- the two are balanced, so the schedule
minimizes DVE start lag and end jam):
  - Image 0 first, loaded in 1MiB quarters with a fold-chain tree: DVE starts
    ~13us in. Image 1 next in halves with a whole-image tree.
  - Images 2..9 as PAIRS, each loaded by ONE 8MiB transfer (per-partition two
    32KB runs at 4MiB stride; fewer transfers avoided a per-transfer SDMA
    straggler penalty seen with 2MiB chunks). Each reduction level is one TT
    per tree with a multi-dim AP spanning both images (12 TTs, ~18.6us vs
    ~21.4us unfused), just under the 19.35us pair DMA cadence.
  - Image 10 in halves, image 11 in quarters with the fold chain, so the DVE
    work that depends on the final DMA chunk is only ~4.6us -> short tail.
  - ACT does ln with free accumulation into per-image/pair columns of parts
    matrices; one subtract/reduce/matmul(ones) collapse at the end gives a
    single-descriptor [1,1] output DMA.
"""
import numpy as np
import concourse.bass as bass
import concourse.mybir as mybir
import concourse.tile as tile
from concourse.bass_utils import run_bass_kernel_spmd

_N_CORES = 8
_B, _C, _H, _W = 32, 3, 1024, 1024
_IMGS_PER_CORE = (_B // _N_CORES) * _C  # 12
_WIN = 8
_EPS = 1e-4

_NC_CACHE = {}
LAST_RESULTS = None  # BassKernelResults of the most recent run (for test.py)


def _split_excess_waits(nc, max_waits=1):
    """This walrus build rejects >2 sync-waits on one CTRL instruction (the
    Tile exit drain collects one wait per active logical proc). Move excess
    waits onto preceding NoOps on the same engine."""
    for func in nc.m.functions:
        for bb in func.blocks:
            insts = bb.instructions
            out_insts = []
            changed = False
            for ins in insts:
                si = getattr(ins, "sync_info", None)
                if si is not None and si.on_wait and len(si.on_wait) > max_waits:
                    waits = list(si.on_wait)
                    head, tail = waits[:-max_waits], waits[-max_waits:]
                    for j in range(0, len(head), max_waits):
                        nop = mybir.InstNoOp(name=f"{ins.name}-wsplit{j}", ins=[], outs=[])
                        nop.engine = ins.engine
                        nop.sync_info = mybir.SyncInfo(
                            on_wait=head[j:j + max_waits], on_update=[])
                        out_insts.append(nop)
                    ins.sync_info = mybir.SyncInfo(on_wait=tail, on_update=si.on_update)
                    changed = True
                out_insts.append(ins)
            if changed:
                bb.instructions = out_insts


def _light_drain_and_barrier(self, tick_clock, wait_clock):
    """TileContext exit ceremony minus the trailing all-engine barrier
    (drain already waits on the global clock; NEFF completion waits on all
    engine programs regardless). Saves a few us of kernel-exit time."""
    from concourse.vector_clock import ScopedClock
    drain_inst = self.nc.sync.drain()
    wait_clock.add_sem_waits(drain_inst.ins,
                             ScopedClock({None: tick_clock.global_clock}))
    self.nc.all_engine_barrier()
    popped = self.nc._tile_sem_poison_stack.pop()
    assert popped is self._sem_poison
    # skip clear_and_free_semaphores: NRT resets engine/sem state per
    # execution, and nothing runs after this context in the program
    self.nc._state.prepend_free_semaphores(
        [s.num if hasattr(s, "num") else s for s in self.sems.allocated().values()])


def _build():
    F32 = mybir.dt.float32
    BF16 = mybir.dt.bfloat16
    MAX = mybir.AluOpType.max
    MIN = mybir.AluOpType.min
    LN = mybir.ActivationFunctionType.Ln
    OPS = ((0, MAX), (1, MIN))  # (tree index, op); tree 0 = max, tree 1 = min

    nc = bass.Bass()
    y = nc.declare_dram_parameter("y", [_IMGS_PER_CORE, _H, _W], F32, isOutput=False)
    out = nc.declare_dram_parameter("out", [1, 1], F32, isOutput=True)

    n_cols = 7  # parts columns: 5 pairs, img10, img11

    tile.TileContext._drain_and_barrier = _light_drain_and_barrier
    with tile.TileContext(nc) as tc:
        with tc.tile_pool(name="pair", bufs=2) as pair_pool, \
             tc.tile_pool(name="taper", bufs=1) as taper_pool, \
             tc.tile_pool(name="work", bufs=1) as work_pool, \
             tc.tile_pool(name="stat", bufs=2) as stat_pool, \
             tc.tile_pool(name="accp", bufs=1) as acc_pool, \
             tc.tile_pool(name="psum", bufs=1, space="PSUM") as psum_pool:
            parts_mx = acc_pool.tile([128, n_cols], F32, tag="pmx")
            parts_mn = acc_pool.tile([128, n_cols], F32, tag="pmn")
            # warm the SP HWDGE queue at kernel start so the final out-DMA
            # doesn't pay first-use latency on the completion semaphore
            warm = acc_pool.tile([1, 1], F32, tag="warm")
            nc.sync.dma_start(out=warm[:], in_=y[0, 0:1, 0:1])

            def load(dst_ap, img, lo, hi):
                src = y[img].rearrange("(p r) c -> p (r c)", p=128)
                nc.gpsimd.dma_start(out=dst_ap, in_=src[:, lo:hi])

            # pair 0 (images 0,1): two 4MiB per-image transfers so L1 can
            # start once image 0 lands (~10us earlier than one 8MiB load)
            T0 = pair_pool.tile([128, 16384], BF16, tag="T")
            for i in range(2):
                load(T0[:, i * 8192:(i + 1) * 8192], i, 0, 8192)

            eps = acc_pool.tile([128, 1], F32, tag="eps")
            nc.gpsimd.memset(eps[:], _EPS)
            ones = acc_pool.tile([128, 1], F32, tag="ones")
            nc.gpsimd.memset(ones[:], 1.0)
            lnscr = acc_pool.tile([128, 256], F32, tag="lnscr")  # ACT out sink

            def ln_accum(src_ap, col, t):
                parts = parts_mx if t == 0 else parts_mn
                nc.scalar.activation(lnscr[:, 0:src_ap.shape[-1]], src_ap, LN,
                                     bias=eps[:], accum_out=parts[:, col:col + 1])

            def hlevels(cur2, seg, res2, t, op):
                """j-direction 8->4->2->1 for one tree. cur2 holds `seg`
                window-column groups of 1024; res2 gets seg*128 results at
                offset t*seg*128. Scratch tags sized for seg=2, sliced."""
                h4 = work_pool.tile([128, 1024], BF16, tag=f"h4t{t}")
                cj = cur2.rearrange("p (s w j) -> p s w j", s=seg, j=8)
                nc.vector.tensor_tensor(
                    out=h4[:, 0:seg * 512].rearrange("p (s w j) -> p s w j",
                                                     s=seg, j=4),
                    in0=cj[:, :, :, 0:4], in1=cj[:, :, :, 4:8], op=op)
                h2 = work_pool.tile([128, 512], BF16, tag=f"h2t{t}")
                hj = h4[:, 0:seg * 512].rearrange("p (s w j) -> p s w j",
                                                  s=seg, j=4)
                nc.vector.tensor_tensor(
                    out=h2[:, 0:seg * 256].rearrange("p (s w j) -> p s w j",
                                                     s=seg, j=2),
                    in0=hj[:, :, :, 0:2], in1=hj[:, :, :, 2:4], op=op)
                rj = h2[:, 0:seg * 256].rearrange("p (s w j) -> p s w j",
                                                  s=seg, j=2)
                nc.vector.tensor_tensor(
                    out=res2[:, t * seg * 128:(t + 1) * seg * 128].rearrange(
                        "p (s w j) -> p s w j", s=seg, j=1),
                    in0=rj[:, :, :, 0:1], in1=rj[:, :, :, 1:2], op=op)

            def fold_img(Ts, res_t, col):
                """Quarter-granular fold-chain tree over Ts [128, 8192]
                (quarters = row pairs): each TT depends on at most one
                quarter plus the running fold."""
                for t, op in OPS:
                    s = work_pool.tile([128, 4096], BF16, tag="scr")
                    f = None
                    for q in range(4):
                        w = s[:, (q % 2) * 1024:((q % 2) + 1) * 1024]
                        nc.vector.tensor_tensor(
                            out=w, in0=Ts[:, q * 2048:q * 2048 + 1024],
                            in1=Ts[:, q * 2048 + 1024:(q + 1) * 2048], op=op)
                        if q == 0:
                            f = w
                        else:
                            nf = s[:, (2 + (q % 2)) * 1024:(3 + (q % 2)) * 1024]
                            nc.vector.tensor_tensor(out=nf, in0=f, in1=w, op=op)
                            f = nf
                    hlevels(f, 1, res_t, t, op)
                    ln_accum(res_t[:, t * 128:(t + 1) * 128], col, t)

            def tree_img(Ts, res_t, col):
                """Whole-image tree over Ts [128, 8192] (halves): L1 per
                half so each depends on one 2MiB chunk."""
                ab = work_pool.tile([128, 16384], BF16, tag="ab")
                for t, op in OPS:
                    base = t * 4096
                    nc.vector.tensor_tensor(out=ab[:, base:base + 2048],
                                            in0=Ts[:, 0:2048],
                                            in1=Ts[:, 2048:4096], op=op)
                    nc.vector.tensor_tensor(out=ab[:, base + 2048:base + 4096],
                                            in0=Ts[:, 4096:6144],
                                            in1=Ts[:, 6144:8192], op=op)
                c = work_pool.tile([128, 8192], BF16, tag="c")
                for t, op in OPS:
                    nc.vector.tensor_tensor(
                        out=c[:, t * 2048:(t + 1) * 2048],
                        in0=ab[:, t * 4096:t * 4096 + 2048],
                        in1=ab[:, t * 4096 + 2048:(t + 1) * 4096], op=op)
                cur = work_pool.tile([128, 4096], BF16, tag="cur")
                for t, op in OPS:
                    nc.vector.tensor_tensor(
                        out=cur[:, t * 1024:(t + 1) * 1024],
                        in0=c[:, t * 2048:t * 2048 + 1024],
                        in1=c[:, t * 2048 + 1024:(t + 1) * 2048], op=op)
                for t, op in OPS:
                    hlevels(cur[:, t * 1024:(t + 1) * 1024], 1, res_t, t, op)
                    ln_accum(res_t[:, t * 128:(t + 1) * 128], col, t)

            def pair_l2plus(ab, col):
                """Levels 2+ for a pair, each one TT per tree spanning both
                images; ab = [mx(i2)(b2)(2048) | mn...]."""
                c = work_pool.tile([128, 8192], BF16, tag="c")  # [mx | mn]
                for t, op in OPS:
                    av = ab[:, t * 8192:(t + 1) * 8192].rearrange(
                        "p (i b k) -> p i b k", i=2, b=2)
                    nc.vector.tensor_tensor(
                        out=c[:, t * 4096:(t + 1) * 4096].rearrange(
                            "p (i k) -> p i k", i=2),
                        in0=av[:, :, 0, :], in1=av[:, :, 1, :], op=op)
                cur = work_pool.tile([128, 4096], BF16, tag="cur")  # [mx | mn]
                for t, op in OPS:
                    cv = c[:, t * 4096:(t + 1) * 4096].rearrange(
                        "p (i k) -> p i k", i=2)
                    nc.vector.tensor_tensor(
                        out=cur[:, t * 2048:(t + 1) * 2048].rearrange(
                            "p (i k) -> p i k", i=2),
                        in0=cv[:, :, 0:1024], in1=cv[:, :, 1024:2048], op=op)
                res = stat_pool.tile([128, 512], BF16, tag="res")  # [mx | mn]
                for t, op in OPS:
                    hlevels(cur[:, t * 2048:(t + 1) * 2048], 2, res, t, op)
                ln_accum(res[:, 0:256], col, 0)
                ln_accum(res[:, 256:512], col, 1)

            # ---- pair 0: L1 per image (each gated on one 4MiB transfer) ----
            ab0 = work_pool.tile([128, 16384], BF16, tag="ab")  # [mx | mn]
            for i in range(2):
                Tv = T0[:, i * 8192:(i + 1) * 8192].rearrange(
                    "p (b k) -> p b k", b=2)
                for t, op in OPS:
                    nc.vector.tensor_tensor(
                        out=ab0[:, t * 8192 + i * 4096:t * 8192 + (i + 1) * 4096]
                        .rearrange("p (b k) -> p b k", b=2),
                        in0=Tv[:, :, 0:2048], in1=Tv[:, :, 2048:4096], op=op)
            pair_l2plus(ab0, 0)

            # ---- images 2..9 as pairs: one 8MiB transfer, 12 fused TTs ----
            for k in range(4):
                T = pair_pool.tile([128, 16384], BF16, tag="T")
                src = y[2 * k + 2:2 * k + 4].rearrange(
                    "i (p r) c -> p i (r c)", p=128)
                nc.gpsimd.dma_start(
                    out=T[:].rearrange("p (i k) -> p i k", i=2), in_=src)
                Tv = T[:].rearrange("p (i b k) -> p i b k", i=2, b=2)
                ab = work_pool.tile([128, 16384], BF16, tag="ab")  # [mx | mn]
                for t, op in OPS:
                    abv = ab[:, t * 8192:(t + 1) * 8192].rearrange(
                        "p (i b k) -> p i b k", i=2, b=2)
                    nc.vector.tensor_tensor(out=abv, in0=Tv[:, :, :, 0:2048],
                                            in1=Tv[:, :, :, 2048:4096], op=op)
                pair_l2plus(ab, 1 + k)

            # ---- images 10 (tree) and 11 (fold) from the back taper tile ----
            Tb = taper_pool.tile([128, 16384], BF16, tag="Tt")
            load(Tb[:, 0:4096], 10, 0, 4096)
            load(Tb[:, 4096:8192], 10, 4096, 8192)
            for q in range(4):
                load(Tb[:, 8192 + q * 2048:8192 + (q + 1) * 2048],
                     11, q * 2048, (q + 1) * 2048)
            res10 = stat_pool.tile([128, 256], BF16, tag="res10")
            tree_img(Tb[:, 0:8192], res10, 5)
            res11 = stat_pool.tile([128, 256], BF16, tag="res11")
            fold_img(Tb[:, 8192:16384], res11, 6)

            # ---- final combine ----
            diff = acc_pool.tile([128, n_cols], F32, tag="diff")
            nc.vector.tensor_tensor(out=diff[:], in0=parts_mx[:], in1=parts_mn[:],
                                    op=mybir.AluOpType.subtract)
            acc = acc_pool.tile([128, 1], F32, tag="acc")
            nc.vector.tensor_reduce(out=acc[:], in_=diff[:],
                                    axis=mybir.AxisListType.X,
                                    op=mybir.AluOpType.add)
            # collapse partitions with a 1x128 @ 128x1 matmul so the out-DMA
            # is a single descriptor
            pt = psum_pool.tile([1, 1], F32, tag="pt")
            nc.tensor.matmul(pt[:], acc[:], ones[:])
            total = acc_pool.tile([1, 1], F32, tag="total")
            nc.vector.tensor_copy(out=total[:], in_=pt[:])
            nc.sync.dma_start(out=out[:], in_=total[:])

    _split_excess_waits(nc)
    return nc


def _get_nc():
    if "nc" not in _NC_CACHE:
        _NC_CACHE["nc"] = _build()
    return _NC_CACHE["nc"]


def kernel(y_pred, winSize=8, _trace=False, **_ignored):
    global LAST_RESULTS
    assert int(winSize) == _WIN
    y = np.ascontiguousarray(np.asarray(y_pred, dtype=np.float32))
    assert y.shape == (_B, _C, _H, _W)
    per_core_b = _B // _N_CORES
    in_maps = [
        {"y": y[c * per_core_b:(c + 1) * per_core_b].reshape(_IMGS_PER_CORE, _H, _W)}
        for c in range(_N_CORES)
    ]
    nc = _get_nc()
    res = run_bass_kernel_spmd(nc, in_maps, list(range(_N_CORES)), trace=_trace)
    LAST_RESULTS = res
    total = np.sum([float(r["out"][0, 0]) for r in res.results])
    val = total * 20.0 * (_WIN * _WIN) / (_H * _W) / _B
    return np.float32(val)
